# revision 14
# baseline (speedup 1.0000x reference)
"""Trainium2 Bass kernel for nn_Attention_24043226923261.

Per-pixel cross-attention: RMSNorm(c) -> kv proj -> softmax over N=8 context
slices with a query shared across the 32x32 spatial grid -> out proj.

Sharding: data-parallel over B=8 across the 8 NeuronCores (core b owns batch
b). Zero collectives.

Host-side weight folding (exact math, same as the 100us baseline):
  - query path qh = silu(emb[q]@w1+b1)@w2+b2 is tiny ([8,512]); dots =
    c_norm @ (w_k @ qh^T), so qh, attn_scale and rms_w fold into a per-core
    [256,8] matrix wq.  k is never materialized.
  - rms_w folds into wv/wq; the per-token rsqrt(mean(c^2)) scale s[t,n] is
    applied on device (k-side inside the softmax logits, v-side folded into
    the unnormalized softmax weights P_n = exp(D_n*s_n)*s_n).
  - out proj computed transposed (out^T = wo^T @ h^T) so the result lands
    channel-major [256, H*W] = the required output layout.

v3 design (from trace analysis of the 100.7us baseline and a 105us v2):
  - ~10us fixed runtime startup; c loads are HBM-bound until ~22us.  The
    softmax denominator Z = sum_n E_n is the ONLY cross-n dependency, so
    everything else (squares, dots, mean, per-n softmax bits, v matmuls,
    weighted products P_n*v_n, partial accumulation) is pipelined PER
    CONTEXT-PAIR behind the DMA stream; 1/Z is applied to the accumulated
    h at the very end (one cheap 2x multiply per token tile).
  - v tiles are computed in [128,1024] two-bank PSUM pairs: one ACT copy +
    one DVE 2x multiply per pair instead of two of each (amortizes the
    fixed psum-access/instruction overheads, which the v2 trace showed
    were 25-40% of op cost).
  - hs axis uses (d,e) order so the per-(e)-broadcast multiply keeps a
    packed stride-1 last dim -> DVE 2x_1p (measured 400ns vs 659 f32).
  - work split: DVE muls/folds, ACT pair-copies + exp/sqrt + ht/bias,
    GPSIMD 2 squares + 8 pair-adds, ~9 direct-from-psum pair-muls on DVE
    to balance ACT; bf16 output halves out-DMA.
"""

import sys

for _p in ("/opt/trn_rl_repo",):
    if _p not in sys.path:
        sys.path.insert(0, _p)

import numpy as np


B = 8
N = 8          # context slices (softmax axis)
NP = N // 2    # context pairs
CH = 256       # channels / hidden
H = W = 32
T = H * W      # 1024 spatial tokens per batch
HEADS = 8
HD = 64        # head dim
HS = HEADS * HD  # 512
EPS = 1e-6
NCORES = 8
PT = 128       # partition tile
TT = T // PT   # 8 token tiles
KCH = CH // PT  # 2 contraction chunks over channels
KHS = HS // PT  # 4 contraction chunks over (d, e)
GRP = 4        # token tiles per out-proj batch

# squares engine per n: v=vector, a=scalar(ACT), g=gpsimd
SQ_ENG = {0: 'v', 1: 'v', 2: 'a', 3: 'a', 4: 'g', 5: 'g', 6: 'v', 7: 'v'}
# (tt, pair) whose P*v multiply reads PSUM f32 directly (no ACT copy);
# chosen to relieve ACT in the middle pairs
DIRECT = {(tt, p) for tt in range(TT) for p in (1, 2) if tt % 2 == 0}
# accumulation adds (pairs 1,2 into acc) routed to GPSIMD for these tts
GPS_ADD_TTS = (1, 3, 5, 7)


def _kernel_body(nc, tc, d):
    from contextlib import ExitStack

    from concourse import mybir

    AF = mybir.ActivationFunctionType
    ALU = mybir.AluOpType
    f32 = mybir.dt.float32
    bf16 = mybir.dt.bfloat16

    with ExitStack() as ctx:
        const = ctx.enter_context(tc.tile_pool(name="const", bufs=1))
        cpool = ctx.enter_context(tc.tile_pool(name="c", bufs=1))
        csqp = ctx.enter_context(tc.tile_pool(name="csq", bufs=2))
        sp = ctx.enter_context(tc.tile_pool(name="s", bufs=1))
        ep = ctx.enter_context(tc.tile_pool(name="e", bufs=1))
        vsb = ctx.enter_context(tc.tile_pool(name="vsb", bufs=3))
        accp = ctx.enter_context(tc.tile_pool(name="acc", bufs=1))
        tmpp = ctx.enter_context(tc.tile_pool(name="tmp", bufs=3))
        hp = ctx.enter_context(tc.tile_pool(name="h", bufs=4))
        htp = ctx.enter_context(tc.tile_pool(name="ht", bufs=2))
        outp = ctx.enter_context(tc.tile_pool(name="o", bufs=2))
        psD = ctx.enter_context(tc.tile_pool(name="psD", bufs=1, space="PSUM"))
        psM = ctx.enter_context(tc.tile_pool(name="psM", bufs=1, space="PSUM"))
        psV = ctx.enter_context(tc.tile_pool(name="psV", bufs=2, space="PSUM"))
        psT = ctx.enter_context(tc.tile_pool(name="psT", bufs=1, space="PSUM"))
        psO = ctx.enter_context(tc.tile_pool(name="psO", bufs=1, space="PSUM"))

        eps_sb = const.tile([PT, 1], f32, tag="eps", name="eps")
        nc.vector.memset(eps_sb[:], EPS)

        # ---- DMA issues over the 3 HWDGE queues ----
        c_sb = {}

        def _load_c(eng, n):
            t = cpool.tile([PT, KCH * T], bf16, tag=f"c{n}", name=f"c{n}")
            for k in range(KCH):
                eng.dma_start(t[:, k * T:(k + 1) * T],
                              d["c"][n, k * PT:(k + 1) * PT, :])
            c_sb[n] = t

        _load_c(nc.sync, 0)
        _load_c(nc.sync, 1)
        wv_sb = []
        for k in range(KCH):
            t = const.tile([PT, HS], bf16, tag=f"wv{k}", name=f"wv{k}")
            nc.sync.dma_start(t[:], d["wv"][k * PT:(k + 1) * PT, :])
            wv_sb.append(t)
        _load_c(nc.sync, 6)
        wo_sb = []
        for k in range(KHS):
            t = const.tile([PT, CH], bf16, tag=f"wo{k}", name=f"wo{k}")
            nc.sync.dma_start(t[:], d["wo"][k * PT:(k + 1) * PT, :])
            wo_sb.append(t)
        bo_sb = []
        for m in range(CH // PT):
            t = const.tile([PT, 1], f32, tag=f"bo{m}", name=f"bo{m}")
            nc.sync.dma_start(t[:], d["bo"][m * PT:(m + 1) * PT, :])
            bo_sb.append(t)
        eye_sb = const.tile([PT, PT], bf16, tag="eye", name="eye")
        nc.sync.dma_start(eye_sb[:], d["eye"][:, :])

        wq_sb = []
        invc_sb = []
        for k in range(KCH):
            t = const.tile([PT, HEADS], bf16, tag=f"wq{k}", name=f"wq{k}")
            nc.scalar.dma_start(t[:], d["wq"][k * PT:(k + 1) * PT, :])
            wq_sb.append(t)
        for k in range(KCH):
            t = const.tile([PT, 1], bf16, tag=f"invc{k}", name=f"invc{k}")
            nc.scalar.dma_start(t[:], d["invc"][k * PT:(k + 1) * PT, :])
            invc_sb.append(t)
        _load_c(nc.scalar, 2)
        _load_c(nc.scalar, 3)
        _load_c(nc.gpsimd, 4)
        _load_c(nc.gpsimd, 5)
        _load_c(nc.gpsimd, 7)

        # ---- persistent pass-0 / softmax state ----
        # D_ps cols (tt, n, e); mean_ps cols (tt, n)
        D_ps = psD.tile([PT, TT * N * HEADS], f32, name="D")
        Dv = D_ps[:].rearrange("p (a n e) -> p a n e", a=TT, n=N)
        mean_ps = psM.tile([PT, TT * N], f32, name="mean")
        mv = mean_ps[:].rearrange("p (a n) -> p a n", n=N)
        sq_all = sp.tile([PT, TT * N], f32, tag="sq", name="sq_all")
        sqv = sq_all[:].rearrange("p (a n) -> p a n", n=N)
        s_all = sp.tile([PT, TT * N], f32, tag="s", name="s_all")
        sv = s_all[:].rearrange("p (a n) -> p a n", n=N)
        Dsc = ep.tile([PT, TT * N * HEADS], bf16, tag="Dsc", name="Dsc")
        Dscv = Dsc[:].rearrange("p (a n e) -> p a n e", a=TT, n=N)
        E = ep.tile([PT, TT * N * HEADS], bf16, tag="E", name="E")
        Ev = E[:].rearrange("p (a n e) -> p a n e", a=TT, n=N)
        P = ep.tile([PT, TT * N * HEADS], bf16, tag="P", name="P")
        Pv = P[:].rearrange("p (a n e) -> p a n e", a=TT, n=N)
        Z = ep.tile([PT, TT * HEADS], f32, tag="Z", name="Z")
        Zv = Z[:].rearrange("p (a e) -> p a e", e=HEADS)

        csq = {}

        def _emit_square(n):
            eng = {'v': nc.vector, 'a': nc.scalar, 'g': nc.gpsimd}[SQ_ENG[n]]
            t = csqp.tile([PT, KCH * T], bf16, tag=f"csq_{SQ_ENG[n]}",
                          name=f"csq{n}")
            if SQ_ENG[n] == 'a':
                nc.scalar.activation(t[:], c_sb[n][:], AF.Square)
            else:
                eng.tensor_mul(t[:], c_sb[n][:], c_sb[n][:])
            csq[n] = t

        def _emit_dots(n):
            for tt in range(TT):
                for k in range(KCH):
                    nc.tensor.matmul(
                        Dv[:, tt, n, :],
                        c_sb[n][:, k * T + tt * PT: k * T + (tt + 1) * PT],
                        wq_sb[k][:],
                        start=(k == 0), stop=(k == KCH - 1),
                    )

        def _emit_mean(n):
            for tt in range(TT):
                for k in range(KCH):
                    nc.tensor.matmul(
                        mv[:, tt, n: n + 1],
                        csq[n][:, k * T + tt * PT: k * T + (tt + 1) * PT],
                        invc_sb[k][:],
                        start=(k == 0), stop=(k == KCH - 1),
                    )

        def _emit_softmax_n(n):
            # s_n = 1/sqrt(mean_n + eps)
            nc.scalar.activation(sqv[:, :, n], mv[:, :, n], AF.Sqrt,
                                 bias=eps_sb[:])
            nc.vector.reciprocal(sv[:, :, n], sqv[:, :, n])
            s_bc = sv[:, :, n].rearrange("p (a o) -> p a o", o=1) \
                              .broadcast_to([PT, TT, HEADS])
            # logits scaled by s_n, exp, unnormalized weights P_n = E_n*s_n
            nc.vector.tensor_mul(Dscv[:, :, n, :], Dv[:, :, n, :], s_bc)
            nc.scalar.activation(Ev[:, :, n, :], Dscv[:, :, n, :], AF.Exp)
            nc.vector.tensor_mul(Pv[:, :, n, :], Ev[:, :, n, :], s_bc)
            # Z accumulation
            if n == 0:
                nc.vector.tensor_copy(Zv, Ev[:, :, n, :])
            else:
                nc.vector.tensor_add(Zv, Zv, Ev[:, :, n, :])

        # ---- v pair pipeline ----
        v_ps = {}
        v_sb = {}
        acc = {}

        def _emit_vpair(tt, p):
            ps = psV.tile([PT, 2 * HS], f32, tag="v", name=f"v{tt}_{p}")
            for nn in range(2):
                n = 2 * p + nn
                for k in range(KCH):
                    nc.tensor.matmul(
                        ps[:, nn * HS:(nn + 1) * HS],
                        c_sb[n][:, k * T + tt * PT: k * T + (tt + 1) * PT],
                        wv_sb[k][:],
                        start=(k == 0), stop=(k == KCH - 1),
                    )
            v_ps[tt, p] = ps

        def _emit_vcopy(tt, p):
            if (tt, p) in DIRECT:
                return
            t = vsb.tile([PT, 2 * HS], bf16, tag="vsb", name=f"vsb{tt}_{p}")
            nc.scalar.copy(t[:], v_ps[tt, p][:])
            v_sb[tt, p] = t

        def _emit_pairmul(tt, p):
            # weight view: (n-pair, d, e) with packed e innermost -> 2x
            w_b = Pv[:, tt, 2 * p:2 * p + 2, :] \
                .rearrange("p n (o e) -> p n o e", o=1) \
                .broadcast_to([PT, 2, HD, HEADS])
            direct = (tt, p) in DIRECT
            src = v_ps[tt, p] if direct else v_sb[tt, p]
            if p == 0:
                tgt = accp.tile([PT, 2 * HS], bf16, tag="acc", bufs=TT,
                                name=f"acc{tt}")
                acc[tt] = tgt
            else:
                tgt = tmpp.tile([PT, 2 * HS], bf16, tag="tmp", name=f"tmp{tt}_{p}")
            nc.vector.tensor_mul(
                tgt[:].rearrange("p (n dd e) -> p n dd e", n=2, e=HEADS),
                src[:].rearrange("p (n dd e) -> p n dd e", n=2, e=HEADS),
                w_b,
            )
            if p > 0:
                eng = nc.gpsimd if (tt in GPS_ADD_TTS and p < 3) else nc.vector
                eng.tensor_add(acc[tt][:], acc[tt][:], tgt[:])

        # ---- finals: fold pair-width acc to h and apply 1/Z ----
        rZ = ep.tile([PT, TT * HEADS], bf16, tag="rZ", name="rZ")

        def _emit_rz():
            with nc.allow_low_precision(reason="softmax weights are bf16"):
                nc.vector.reciprocal(rZ[:], Z[:])

        h_tiles = {}

        def _emit_final(tt):
            hu = hp.tile([PT, HS], bf16, tag="hu", name=f"hu{tt}")
            nc.vector.tensor_add(hu[:], acc[tt][:, 0:HS], acc[tt][:, HS:2 * HS])
            rZ_b = rZ[:].rearrange("p (a e) -> p a e", e=HEADS)[:, tt, :] \
                .rearrange("p (o e) -> p o e", o=1) \
                .broadcast_to([PT, HD, HEADS])
            h = hp.tile([PT, HS], bf16, tag="h", name=f"h{tt}")
            nc.vector.tensor_mul(
                h[:].rearrange("p (dd e) -> p dd e", e=HEADS),
                hu[:].rearrange("p (dd e) -> p dd e", e=HEADS),
                rZ_b,
            )
            h_tiles[tt] = h

        # ---- transpose + out-proj ----
        ht_sb = {}
        tr_done = {}

        def _emit_transposes(tt):
            g = tt // GRP
            if tt % GRP == 0:
                ht_sb[g] = htp.tile([PT, KHS * GRP * PT], bf16, tag="ht",
                                    name=f"ht{g}")
            tr = psT.tile([PT, KHS * PT], bf16, tag="tr", bufs=1,
                          name=f"tr{tt}")
            h = h_tiles[tt]
            for m in range(KHS):
                nc.tensor.transpose(tr[:, m * PT:(m + 1) * PT],
                                    h[:, m * PT:(m + 1) * PT], eye_sb[:])
            tr_done[tt] = tr

        def _emit_htcopy(tt):
            g = tt // GRP
            out_view = ht_sb[g][:].rearrange(
                "p (m q c) -> p m q c", m=KHS, q=GRP)[:, :, tt % GRP, :]
            nc.scalar.copy(
                out_view,
                tr_done[tt][:].rearrange("p (m c) -> p m c", m=KHS))

        def _emit_outproj(g):
            for m2 in range(CH // PT):
                o_ps = psO.tile([PT, GRP * PT], f32, tag="o",
                                name=f"ops{g}_{m2}")
                for k in range(KHS):
                    nc.tensor.matmul(
                        o_ps[:],
                        wo_sb[k][:, m2 * PT:(m2 + 1) * PT],
                        ht_sb[g][:, k * GRP * PT:(k + 1) * GRP * PT],
                        start=(k == 0), stop=(k == KHS - 1),
                    )
                o_sb = outp.tile([PT, GRP * PT], bf16, tag="osb",
                                 name=f"osb{g}_{m2}")
                nc.scalar.activation(o_sb[:], o_ps[:], AF.Identity,
                                     bias=bo_sb[m2][:])
                nc.sync.dma_start(
                    d["out"][m2 * PT:(m2 + 1) * PT,
                             g * GRP * PT:(g + 1) * GRP * PT],
                    o_sb[:])

        # ---- emission schedule: pipelined per context pair ----
        for p in range(NP):
            n0, n1 = 2 * p, 2 * p + 1
            # squares first (gate the means), then dots (only need c),
            # first two v pairs cover the square->mean latency on PE
            _emit_square(n0)
            _emit_square(n1)
            _emit_dots(n0)
            _emit_dots(n1)
            _emit_vpair(0, p)
            _emit_vcopy(0, p)
            _emit_vpair(1, p)
            _emit_vcopy(1, p)
            _emit_mean(n0)
            _emit_mean(n1)
            _emit_softmax_n(n0)
            _emit_softmax_n(n1)
            _emit_pairmul(0, p)
            _emit_pairmul(1, p)
            for tt in range(2, TT):
                _emit_vpair(tt, p)
                _emit_vcopy(tt, p)
                _emit_pairmul(tt, p)
        # tail: 1/Z, then per-tt fold/normalize -> transpose -> out proj
        _emit_rz()
        for tt in range(TT):
            _emit_final(tt)
            _emit_transposes(tt)
            _emit_htcopy(tt)
            if tt % GRP == GRP - 1:
                _emit_outproj(tt // GRP)


def _build_nc():
    import concourse.tile as tile
    from concourse import bacc, mybir

    f32 = mybir.dt.float32
    bf16 = mybir.dt.bfloat16
    nc = bacc.Bacc(
        "TRN2",
        target_bir_lowering=False,
        debug=False,
        enable_asserts=False,
        num_devices=NCORES,
    )
    d = {
        "c": nc.dram_tensor("c", [N, CH, T], bf16, kind="ExternalInput").ap(),
        "wv": nc.dram_tensor("wv", [CH, HS], bf16, kind="ExternalInput").ap(),
        "wq": nc.dram_tensor("wq", [CH, HEADS], bf16,
                             kind="ExternalInput").ap(),
        "wo": nc.dram_tensor("wo", [HS, CH], bf16, kind="ExternalInput").ap(),
        "bo": nc.dram_tensor("bo", [CH, 1], f32, kind="ExternalInput").ap(),
        "invc": nc.dram_tensor("invc", [CH, 1], bf16,
                               kind="ExternalInput").ap(),
        "eye": nc.dram_tensor("eye", [PT, PT], bf16, kind="ExternalInput").ap(),
        "out": nc.dram_tensor("out", [CH, T], bf16, kind="ExternalOutput").ap(),
    }
    with tile.TileContext(nc) as tc:
        _kernel_body(nc, tc, d)
    nc.compile()
    return nc


_NC_CACHE = None


def _get_nc():
    global _NC_CACHE
    if _NC_CACHE is None:
        _NC_CACHE = _build_nc()
    return _NC_CACHE


def _make_in_maps(q, c, rms_w, emb, w1, b1, w2, b2, w_kv, w_out, b_out):
    q = np.asarray(q).astype(np.int64)
    c = np.asarray(c, dtype=np.float32)
    rms_w = np.asarray(rms_w, dtype=np.float32)
    emb = np.asarray(emb, dtype=np.float32)
    w1 = np.asarray(w1, dtype=np.float32)
    b1 = np.asarray(b1, dtype=np.float32)
    w2 = np.asarray(w2, dtype=np.float32)
    b2 = np.asarray(b2, dtype=np.float32)
    w_kv = np.asarray(w_kv, dtype=np.float32)
    w_out = np.asarray(w_out, dtype=np.float32)
    b_out = np.asarray(b_out, dtype=np.float32)

    # query path (tiny: 8 vectors), exact fp32 math as the reference
    qe = emb[q]                                   # [B, CH]
    x1 = qe @ w1 + b1
    h1 = x1 * (1.0 / (1.0 + np.exp(-x1)))         # silu
    qh = (h1 @ w2 + b2).reshape(B, HEADS, HD)

    wkv3 = w_kv.reshape(CH, HEADS, 2 * HD)
    w_k = wkv3[:, :, :HD]                         # [CH, HEADS, HD]
    w_v = wkv3[:, :, HD:]
    wv = (rms_w[:, None, None] * w_v)             # [CH, HEADS, HD]
    # (d, e) column order: col d*HEADS+e
    wv_de = np.ascontiguousarray(
        wv.transpose(0, 2, 1).reshape(CH, HS), dtype=np.float32)
    scale = float(HD) ** -0.5
    # wq[b, ch, e] = rms_w[ch] * scale * sum_d w_k[ch, e, d] * qh[b, e, d]
    wq_all = np.einsum("ced,bed->bce", w_k, qh).astype(np.float32)
    wq_all = wq_all * (scale * rms_w[None, :, None])

    # out proj rows reordered to (d, e): row d*HEADS+e was row e*HD+d
    wo_de = np.ascontiguousarray(
        w_out.reshape(HEADS, HD, CH).transpose(1, 0, 2).reshape(HS, CH),
        dtype=np.float32)

    import ml_dtypes
    bf = ml_dtypes.bfloat16
    shared = {
        "wv": wv_de.astype(bf),
        "wo": wo_de.astype(bf),
        "bo": np.ascontiguousarray(b_out.reshape(CH, 1), dtype=np.float32),
        "invc": np.full((CH, 1), 1.0 / CH, dtype=np.float32).astype(bf),
        "eye": np.eye(PT, dtype=np.float32).astype(bf),
    }
    in_maps = []
    for b in range(B):
        m = dict(shared)
        m["c"] = np.ascontiguousarray(c[b].reshape(N, CH, T)).astype(bf)
        m["wq"] = np.ascontiguousarray(wq_all[b]).astype(bf)
        in_maps.append(m)
    return in_maps


def _run(in_maps, **kwargs):
    from concourse import bass_utils

    nc = _get_nc()
    return bass_utils.run_bass_kernel_spmd(
        nc, in_maps, core_ids=list(range(NCORES)), **kwargs)


def kernel(q, c, rms_w, emb, w1, b1, w2, b2, w_kv, w_out, b_out):
    in_maps = _make_in_maps(q, c, rms_w, emb, w1, b1, w2, b2, w_kv, w_out,
                            b_out)
    res = _run(in_maps)
    outs = [np.asarray(res.results[b]["out"]).astype(np.float32)
            .reshape(CH, H, W) for b in range(B)]
    return np.stack(outs, axis=0)


# revision 17
# speedup vs baseline: 1.0553x; 1.0553x over previous
"""Trainium2 Bass kernel for nn_Attention_24043226923261.

Per-pixel cross-attention: RMSNorm(c) -> kv proj -> softmax over N=8 context
slices with a query shared across the 32x32 spatial grid -> out proj.

Sharding: data-parallel over B=8 across the 8 NeuronCores (core b owns batch
b). Zero collectives.

Host-side weight folding (exact math, as the 100us baseline):
  - query path qh = silu(emb[q]@w1+b1)@w2+b2 is tiny ([8,512]); dots =
    c_norm @ (w_k @ qh^T), so qh, attn_scale and rms_w fold into a per-core
    [256,8] matrix wq.  k is never materialized.
  - rms_w folds into wv/wq; the per-token rsqrt(mean(c^2)) scale s[t,n] is
    applied on device (k-side in the logits, v-side in the softmax weights).
  - out proj computed transposed (out^T = wo^T @ h^T) -> channel-major
    [256, H*W] output layout directly.

v4 (from traces of the baseline and two failed restructures):
  - The combine h = sum_n a_n*v_n was the bottleneck: 64 DVE multiplies at
    ~660ns reading PSUM f32.  Now v is computed in [128,1024] two-bank PSUM
    pairs, ACT copies each pair to SBUF bf16 (~1030ns) during the otherwise
    idle load window, and the weighted multiply runs in DVE 2x_1p mode
    (~640ns per PAIR, i.e. half).  hs uses (d,e) order so the broadcast
    weight view keeps a packed stride-1 last dim (2x requirement).
  - ACT makes room by moving all squares to DVE/GPSIMD; exp is emitted
    before the last copy batch so softmax is not stuck behind it.
  - av layout is (tt, n, e) so pair-slices of the weights are legal views.
  - pair adds: first add per tile on GPSIMD, rest on DVE; folds alternate.
  - bf16 output (rel-err budget allows), halving out-DMA.
  - measured-worse ideas (do not revisit blindly): whole-tt GPSIMD add
    chains (SBUF contention halves concurrent DVE ops), per-n softmax
    pipelining with cross-engine ladders (sqrt->recip->Dsc->exp per n
    stalls every queue), direct-from-psum pair muls mid-stream (block psum
    ring slots until av exists, stalling the PE v-pipe).
"""

import sys

for _p in ("/opt/trn_rl_repo",):
    if _p not in sys.path:
        sys.path.insert(0, _p)

import numpy as np


B = 8
N = 8          # context slices (softmax axis)
NP = N // 2    # context pairs
CH = 256       # channels / hidden
H = W = 32
T = H * W      # 1024 spatial tokens per batch
HEADS = 8
HD = 64        # head dim
HS = HEADS * HD  # 512
EPS = 1e-6
NCORES = 8
PT = 128       # partition tile
TT = T // PT   # 8 token tiles
KCH = CH // PT  # 2 contraction chunks over channels
KHS = HS // PT  # 4 contraction chunks over (d, e)
GRP = 4        # token tiles per out-proj batch

SQ_GPS = (4, 6)          # squares on GPSIMD; all others on DVE
FOLD_GPS = (0, 2, 4)     # acc fold tts on GPSIMD


def _kernel_body(nc, tc, d):
    from contextlib import ExitStack

    from concourse import mybir

    AF = mybir.ActivationFunctionType
    ALU = mybir.AluOpType
    AX = mybir.AxisListType
    f32 = mybir.dt.float32
    bf16 = mybir.dt.bfloat16

    with ExitStack() as ctx:
        const = ctx.enter_context(tc.tile_pool(name="const", bufs=1))
        cpool = ctx.enter_context(tc.tile_pool(name="c", bufs=1))
        csqp = ctx.enter_context(tc.tile_pool(name="csq", bufs=2))
        sp = ctx.enter_context(tc.tile_pool(name="s", bufs=1))
        ep = ctx.enter_context(tc.tile_pool(name="e", bufs=1))
        vsb = ctx.enter_context(tc.tile_pool(name="vsb", bufs=32))
        accp = ctx.enter_context(tc.tile_pool(name="acc", bufs=8))
        tmpp = ctx.enter_context(tc.tile_pool(name="tmp", bufs=3))
        hp = ctx.enter_context(tc.tile_pool(name="h", bufs=3))
        htp = ctx.enter_context(tc.tile_pool(name="ht", bufs=2))
        outp = ctx.enter_context(tc.tile_pool(name="o", bufs=2))
        psD = ctx.enter_context(tc.tile_pool(name="psD", bufs=1, space="PSUM"))
        psV = ctx.enter_context(tc.tile_pool(name="psV", bufs=2, space="PSUM"))
        psT = ctx.enter_context(tc.tile_pool(name="psT", bufs=1, space="PSUM"))
        psO = ctx.enter_context(tc.tile_pool(name="psO", bufs=2, space="PSUM"))

        eps_sb = const.tile([PT, 1], f32, tag="eps", name="eps")
        nc.vector.memset(eps_sb[:], EPS)

        # ---- DMA issues ----
        c_sb = {}

        def _load_c(eng, n):
            t = cpool.tile([PT, KCH * T], bf16, tag=f"c{n}", name=f"c{n}")
            for k in range(KCH):
                eng.dma_start(t[:, k * T:(k + 1) * T],
                              d["c"][n, k * PT:(k + 1) * PT, :])
            c_sb[n] = t

        wq_sb = []
        invc_sb = []
        for k in range(KCH):
            t = const.tile([PT, HEADS], bf16, tag=f"wq{k}", name=f"wq{k}")
            nc.sync.dma_start(t[:], d["wq"][k * PT:(k + 1) * PT, :])
            wq_sb.append(t)
        for k in range(KCH):
            t = const.tile([PT, 1], bf16, tag=f"invc{k}", name=f"invc{k}")
            nc.sync.dma_start(t[:], d["invc"][k * PT:(k + 1) * PT, :])
            invc_sb.append(t)
        _load_c(nc.sync, 0)
        wv_sb = []
        for k in range(KCH):
            t = const.tile([PT, HS], bf16, tag=f"wv{k}", name=f"wv{k}")
            nc.sync.dma_start(t[:], d["wv"][k * PT:(k + 1) * PT, :])
            wv_sb.append(t)
        _load_c(nc.gpsimd, 1)
        _load_c(nc.gpsimd, 2)
        for n in range(3, N):
            _load_c(nc.sync, n)
        wo_sb = []
        for k in range(KHS):
            t = const.tile([PT, CH], bf16, tag=f"wo{k}", name=f"wo{k}")
            nc.sync.dma_start(t[:], d["wo"][k * PT:(k + 1) * PT, :])
            wo_sb.append(t)
        bo_sb = []
        for m in range(CH // PT):
            t = const.tile([PT, 1], f32, tag=f"bo{m}", name=f"bo{m}")
            nc.sync.dma_start(t[:], d["bo"][m * PT:(m + 1) * PT, :])
            bo_sb.append(t)
        eye_sb = const.tile([PT, PT], bf16, tag="eye", name="eye")
        nc.sync.dma_start(eye_sb[:], d["eye"][:, :])

        # ---- persistent state ----
        # D_ps cols (tt, n, e) so the weight pair-slices are legal views
        D_ps = psD.tile([PT, TT * N * HEADS], f32, name="D")
        Dv = D_ps[:].rearrange("p (a n e) -> p a n e", a=TT, n=N)
        sq_all = sp.tile([PT, TT * N], f32, tag="sq", name="sq_all")
        sqv = sq_all[:].rearrange("p (a n) -> p a n", n=N)
        s_all = sp.tile([PT, TT * N], f32, tag="s", name="s_all")

        csq = {}
        v_ps = {}
        v_sb = {}

        def _emit_square(n):
            eng = nc.gpsimd if n in SQ_GPS else nc.vector
            t = csqp.tile([PT, KCH * T], bf16,
                          tag="csq_g" if n in SQ_GPS else "csq_v",
                          name=f"csq{n}")
            eng.tensor_mul(t[:], c_sb[n][:], c_sb[n][:])
            csq[n] = t

        def _emit_dots(n):
            for tt in range(TT):
                for k in range(KCH):
                    nc.tensor.matmul(
                        Dv[:, tt, n, :],
                        c_sb[n][:, k * T + tt * PT: k * T + (tt + 1) * PT],
                        wq_sb[k][:],
                        start=(k == 0), stop=(k == KCH - 1),
                    )

        def _emit_mean_sqrt(n):
            mean_ps = psO.tile([PT, TT], f32, tag="o", name=f"mean{n}")
            for tt in range(TT):
                for k in range(KCH):
                    nc.tensor.matmul(
                        mean_ps[:, tt:tt + 1],
                        csq[n][:, k * T + tt * PT: k * T + (tt + 1) * PT],
                        invc_sb[k][:],
                        start=(k == 0), stop=(k == KCH - 1),
                    )
            nc.scalar.activation(sqv[:, :, n], mean_ps[:], AF.Sqrt,
                                 bias=eps_sb[:])

        def _emit_vpair(tt, p):
            ps = psV.tile([PT, 2 * HS], f32, tag="v", name=f"v{tt}_{p}")
            for nn in range(2):
                n = 2 * p + nn
                for k in range(KCH):
                    nc.tensor.matmul(
                        ps[:, nn * HS:(nn + 1) * HS],
                        c_sb[n][:, k * T + tt * PT: k * T + (tt + 1) * PT],
                        wv_sb[k][:],
                        start=(k == 0), stop=(k == KCH - 1),
                    )
            v_ps[tt, p] = ps

        def _emit_vcopy(tt, p):
            t = vsb.tile([PT, 2 * HS], bf16, tag="vsb", name=f"vsb{tt}_{p}")
            nc.scalar.copy(t[:], v_ps[tt, p][:])
            v_sb[tt, p] = t

        # ---- softmax head (merged full-width, (tt, n, e) layout) ----
        Dsc = ep.tile([PT, TT * N * HEADS], bf16, tag="Dsc", name="Dsc")
        E = ep.tile([PT, TT * N * HEADS], bf16, tag="E", name="E")

        def _emit_softmax_pre():
            # s = 1/sqrt(mean+eps); logits * s; exp
            nc.vector.reciprocal(s_all[:], sq_all[:])
            s_bc = s_all[:].rearrange("p (a n o) -> p a n o", n=N, o=1) \
                           .broadcast_to([PT, TT, N, HEADS])
            nc.vector.tensor_mul(
                Dsc[:].rearrange("p (a n e) -> p a n e", a=TT, n=N), Dv, s_bc)
            nc.scalar.activation(E[:], Dsc[:], AF.Exp)

        # ---- pass 0 + v pipeline, per context slice ----
        for n in range(N):
            _emit_square(n)
            _emit_dots(n)
            _emit_mean_sqrt(n)
            if n % 2 == 1:
                p = n // 2
                if p == 3:
                    # emit exp ahead of the last copy batch so softmax is
                    # not queued behind it on ACT (E gated on Dsc anyway)
                    _emit_softmax_pre()
                for tt in range(TT):
                    _emit_vpair(tt, p)
                    _emit_vcopy(tt, p)

        Z = ep.tile([PT, TT * HEADS], f32, tag="Z", name="Z")
        nc.vector.tensor_reduce(
            Z[:], E[:].rearrange("p (a n e) -> p a e n", a=TT, n=N),
            axis=AX.X, op=ALU.add)
        rZ = ep.tile([PT, TT * HEADS], bf16, tag="rZ", name="rZ")
        with nc.allow_low_precision(reason="softmax weights are bf16 anyway"):
            nc.vector.reciprocal(rZ[:], Z[:])
        rZ_bc = rZ[:].rearrange("p (a o e) -> p a o e", o=1, e=HEADS) \
                     .broadcast_to([PT, TT, N, HEADS])
        av1 = ep.tile([PT, TT * N * HEADS], bf16, tag="av1", name="av1")
        nc.vector.tensor_mul(
            av1[:].rearrange("p (a n e) -> p a n e", a=TT, n=N),
            E[:].rearrange("p (a n e) -> p a n e", a=TT, n=N), rZ_bc)
        av = ep.tile([PT, TT * N * HEADS], bf16, tag="av", name="av")
        s_bc2 = s_all[:].rearrange("p (a n o) -> p a n o", n=N, o=1) \
                        .broadcast_to([PT, TT, N, HEADS])
        nc.gpsimd.tensor_mul(
            av[:].rearrange("p (a n e) -> p a n e", a=TT, n=N),
            av1[:].rearrange("p (a n e) -> p a n e", a=TT, n=N), s_bc2)
        avv = av[:].rearrange("p (a n e) -> p a n e", a=TT, n=N)

        # ---- combine + transpose + out-proj ----
        ht_sb = {}

        def _emit_combine(tt):
            acc = accp.tile([PT, 2 * HS], bf16, tag="acc", name=f"acc{tt}")
            for p in range(NP):
                w_b = avv[:, tt, 2 * p:2 * p + 2, :] \
                    .rearrange("p n (o e) -> p n o e", o=1) \
                    .broadcast_to([PT, 2, HD, HEADS])
                tgt = acc if p == 0 else tmpp.tile(
                    [PT, 2 * HS], bf16, tag="tmp", name=f"tmp{tt}_{p}")
                nc.vector.tensor_mul(
                    tgt[:].rearrange("p (n dd e) -> p n dd e", n=2, e=HEADS),
                    v_sb[tt, p][:].rearrange("p (n dd e) -> p n dd e",
                                             n=2, e=HEADS),
                    w_b,
                )
                if p > 0:
                    eng = nc.gpsimd if (p == 1 and tt != TT - 1) else nc.vector
                    eng.tensor_add(acc[:], acc[:], tgt[:])
            h = hp.tile([PT, HS], bf16, tag="h", name=f"h{tt}")
            eng = nc.gpsimd if tt in FOLD_GPS else nc.vector
            eng.tensor_add(h[:], acc[:, 0:HS], acc[:, HS:2 * HS])
            return h

        def _emit_tr_out(tt, h):
            g = tt // GRP
            if tt % GRP == 0:
                ht_sb[g] = htp.tile([PT, KHS * GRP * PT], bf16, tag="ht",
                                    name=f"ht{g}")
            tr = psT.tile([PT, KHS * PT], bf16, tag="tr", name=f"tr{tt}")
            for m in range(KHS):
                nc.tensor.transpose(tr[:, m * PT:(m + 1) * PT],
                                    h[:, m * PT:(m + 1) * PT], eye_sb[:])
            out_view = ht_sb[g][:].rearrange(
                "p (m q c) -> p m q c", m=KHS, q=GRP)[:, :, tt % GRP, :]
            nc.scalar.copy(out_view,
                           tr[:].rearrange("p (m c) -> p m c", m=KHS))
            if tt % GRP != GRP - 1:
                return
            for m2 in range(CH // PT):
                o_ps = psO.tile([PT, GRP * PT], f32, tag="o",
                                name=f"ops{g}_{m2}")
                for k in range(KHS):
                    nc.tensor.matmul(
                        o_ps[:],
                        wo_sb[k][:, m2 * PT:(m2 + 1) * PT],
                        ht_sb[g][:, k * GRP * PT:(k + 1) * GRP * PT],
                        start=(k == 0), stop=(k == KHS - 1),
                    )
                o_sb = outp.tile([PT, GRP * PT], bf16, tag="osb",
                                 name=f"osb{g}_{m2}")
                nc.scalar.activation(o_sb[:], o_ps[:], AF.Identity,
                                     bias=bo_sb[m2][:])
                nc.sync.dma_start(
                    d["out"][m2 * PT:(m2 + 1) * PT,
                             g * GRP * PT:(g + 1) * GRP * PT],
                    o_sb[:])

        for tt in range(TT):
            h = _emit_combine(tt)
            _emit_tr_out(tt, h)


def _build_nc():
    import concourse.tile as tile
    from concourse import bacc, mybir

    f32 = mybir.dt.float32
    bf16 = mybir.dt.bfloat16
    nc = bacc.Bacc(
        "TRN2",
        target_bir_lowering=False,
        debug=False,
        enable_asserts=False,
        num_devices=NCORES,
    )
    d = {
        "c": nc.dram_tensor("c", [N, CH, T], bf16, kind="ExternalInput").ap(),
        "wv": nc.dram_tensor("wv", [CH, HS], bf16, kind="ExternalInput").ap(),
        "wq": nc.dram_tensor("wq", [CH, HEADS], bf16,
                             kind="ExternalInput").ap(),
        "wo": nc.dram_tensor("wo", [HS, CH], bf16, kind="ExternalInput").ap(),
        "bo": nc.dram_tensor("bo", [CH, 1], f32, kind="ExternalInput").ap(),
        "invc": nc.dram_tensor("invc", [CH, 1], bf16,
                               kind="ExternalInput").ap(),
        "eye": nc.dram_tensor("eye", [PT, PT], bf16, kind="ExternalInput").ap(),
        "out": nc.dram_tensor("out", [CH, T], bf16, kind="ExternalOutput").ap(),
    }
    with tile.TileContext(nc) as tc:
        _kernel_body(nc, tc, d)
    nc.compile()
    return nc


_NC_CACHE = None


def _get_nc():
    global _NC_CACHE
    if _NC_CACHE is None:
        _NC_CACHE = _build_nc()
    return _NC_CACHE


def _make_in_maps(q, c, rms_w, emb, w1, b1, w2, b2, w_kv, w_out, b_out):
    q = np.asarray(q).astype(np.int64)
    c = np.asarray(c, dtype=np.float32)
    rms_w = np.asarray(rms_w, dtype=np.float32)
    emb = np.asarray(emb, dtype=np.float32)
    w1 = np.asarray(w1, dtype=np.float32)
    b1 = np.asarray(b1, dtype=np.float32)
    w2 = np.asarray(w2, dtype=np.float32)
    b2 = np.asarray(b2, dtype=np.float32)
    w_kv = np.asarray(w_kv, dtype=np.float32)
    w_out = np.asarray(w_out, dtype=np.float32)
    b_out = np.asarray(b_out, dtype=np.float32)

    # query path (tiny: 8 vectors), exact fp32 math as the reference
    qe = emb[q]                                   # [B, CH]
    x1 = qe @ w1 + b1
    h1 = x1 * (1.0 / (1.0 + np.exp(-x1)))         # silu
    qh = (h1 @ w2 + b2).reshape(B, HEADS, HD)

    wkv3 = w_kv.reshape(CH, HEADS, 2 * HD)
    w_k = wkv3[:, :, :HD]                         # [CH, HEADS, HD]
    w_v = wkv3[:, :, HD:]
    wv = (rms_w[:, None, None] * w_v)             # [CH, HEADS, HD]
    # (d, e) column order: col d*HEADS+e
    wv_de = np.ascontiguousarray(
        wv.transpose(0, 2, 1).reshape(CH, HS), dtype=np.float32)
    scale = float(HD) ** -0.5
    # wq[b, ch, e] = rms_w[ch] * scale * sum_d w_k[ch, e, d] * qh[b, e, d]
    wq_all = np.einsum("ced,bed->bce", w_k, qh).astype(np.float32)
    wq_all = wq_all * (scale * rms_w[None, :, None])

    # out proj rows reordered to (d, e): row d*HEADS+e was row e*HD+d
    wo_de = np.ascontiguousarray(
        w_out.reshape(HEADS, HD, CH).transpose(1, 0, 2).reshape(HS, CH),
        dtype=np.float32)

    import ml_dtypes
    bf = ml_dtypes.bfloat16
    shared = {
        "wv": wv_de.astype(bf),
        "wo": wo_de.astype(bf),
        "bo": np.ascontiguousarray(b_out.reshape(CH, 1), dtype=np.float32),
        "invc": np.full((CH, 1), 1.0 / CH, dtype=np.float32).astype(bf),
        "eye": np.eye(PT, dtype=np.float32).astype(bf),
    }
    in_maps = []
    for b in range(B):
        m = dict(shared)
        m["c"] = np.ascontiguousarray(c[b].reshape(N, CH, T)).astype(bf)
        m["wq"] = np.ascontiguousarray(wq_all[b]).astype(bf)
        in_maps.append(m)
    return in_maps


def _run(in_maps, **kwargs):
    from concourse import bass_utils

    nc = _get_nc()
    return bass_utils.run_bass_kernel_spmd(
        nc, in_maps, core_ids=list(range(NCORES)), **kwargs)


def kernel(q, c, rms_w, emb, w1, b1, w2, b2, w_kv, w_out, b_out):
    in_maps = _make_in_maps(q, c, rms_w, emb, w1, b1, w2, b2, w_kv, w_out,
                            b_out)
    res = _run(in_maps)
    outs = [np.asarray(res.results[b]["out"]).astype(np.float32)
            .reshape(CH, H, W) for b in range(B)]
    return np.stack(outs, axis=0)
